# revision 1
# baseline (speedup 1.0000x reference)
"""Trainium2 Bass kernel for CustomTransformerEncoderMoELayer (moe_routing).

Sharding: 8 cores = 2 batches x 4 query-quarters. Each core:
  - projects K^T, V for its full batch (replicated within batch group),
  - computes attention rows for its 256 query tokens (z-score + softmax),
  - residual + LN1, then dense 4-expert MoE (top-2 combine weights) on its
    256 tokens, residual + LN2.
No cross-core communication; host only shards inputs / concatenates outputs.
"""
import os
import numpy as np

import concourse.bacc as bacc
import concourse.mybir as mybir
import concourse.tile as tile
from concourse.bass_utils import run_bass_kernel_spmd

F32 = mybir.dt.float32
F32R = mybir.dt.float32r
AF = mybir.ActivationFunctionType
ALU = mybir.AluOpType
AX = mybir.AxisListType

B, T, D, FFD, E, H = 2, 1024, 768, 3072, 4, 12
HD = D // H          # 64
QTOK = 256           # query tokens per core
NCORES = 8
DC = D // 128        # 6 chunks of contraction dim
FT = FFD // 128      # 24 FF tiles
NTT = QTOK // 128    # 2 token tiles
KB = T // 128        # 8 key blocks
EPS = 1e-5

_cache = {}
LAST_RESULT = None


def _build(gamma: float):
    nc = bacc.Bacc("TRN2", target_bir_lowering=False, debug=False,
                   num_devices=NCORES)

    # ---- DRAM I/O ----
    d_srcT = nc.dram_tensor("srcT", [D, T], F32R, kind="ExternalInput")
    d_srcq = nc.dram_tensor("srcq", [QTOK, D], F32, kind="ExternalInput")
    d_wqT = nc.dram_tensor("wqT", [D, D], F32R, kind="ExternalInput")
    d_wkT = nc.dram_tensor("wkT", [D, D], F32R, kind="ExternalInput")
    d_wvT = nc.dram_tensor("wvT", [D, D], F32R, kind="ExternalInput")
    d_woT = nc.dram_tensor("woT", [D, D], F32R, kind="ExternalInput")
    d_bqc = nc.dram_tensor("bqc", [128, DC], F32, kind="ExternalInput")
    d_bkc = nc.dram_tensor("bkc", [128, DC], F32, kind="ExternalInput")
    d_bvr = nc.dram_tensor("bvr", [1, D], F32R, kind="ExternalInput")
    d_bor = nc.dram_tensor("bor", [1, D], F32R, kind="ExternalInput")
    d_ln1g = nc.dram_tensor("ln1g", [1, D], F32, kind="ExternalInput")
    d_ln1b = nc.dram_tensor("ln1b", [1, D], F32, kind="ExternalInput")
    d_ln2g = nc.dram_tensor("ln2g", [1, D], F32, kind="ExternalInput")
    d_ln2b = nc.dram_tensor("ln2b", [1, D], F32, kind="ExternalInput")
    d_wgT = nc.dram_tensor("wgT", [D, E], F32, kind="ExternalInput")
    d_bgr = nc.dram_tensor("bgr", [1, E], F32R, kind="ExternalInput")
    d_w1 = nc.dram_tensor("w1", [E, D, FFD], F32R, kind="ExternalInput")
    d_b1c = nc.dram_tensor("b1c", [E, 128, FT], F32, kind="ExternalInput")
    d_w2 = nc.dram_tensor("w2", [E, FFD, D], F32R, kind="ExternalInput")
    d_b2r = nc.dram_tensor("b2r", [1, E, D], F32R, kind="ExternalInput")
    d_ident = nc.dram_tensor("ident", [128, 128], F32, kind="ExternalInput")
    d_ones = nc.dram_tensor("ones_r", [1, 128], F32R, kind="ExternalInput")
    d_out = nc.dram_tensor("out", [QTOK, D], F32, kind="ExternalOutput")

    chunks = [(0, 512), (512, 768)]  # free-dim chunks of D for matmul N<=512

    with tile.TileContext(nc) as tc:
        # ----- persistent pools -----
        cpool = tc.alloc_tile_pool(name="const", bufs=1)
        ident = cpool.tile([128, 128], F32)
        nc.sync.dma_start(ident[:], d_ident.ap()[:])
        ones1 = cpool.tile([1, 128], F32R)
        nc.sync.dma_start(ones1[:], d_ones.ap()[:])
        bqc = cpool.tile([128, DC], F32)
        nc.sync.dma_start(bqc[:], d_bqc.ap()[:])
        bkc = cpool.tile([128, DC], F32)
        nc.sync.dma_start(bkc[:], d_bkc.ap()[:])
        bvr = cpool.tile([1, D], F32R)
        nc.sync.dma_start(bvr[:], d_bvr.ap()[:])
        bor = cpool.tile([1, D], F32R)
        nc.sync.dma_start(bor[:], d_bor.ap()[:])
        bgr = cpool.tile([1, E], F32R)
        nc.sync.dma_start(bgr[:], d_bgr.ap()[:])
        b1c = cpool.tile([128, E, FT], F32)
        nc.sync.dma_start(b1c[:], d_b1c.ap().rearrange("e p c -> p e c"))
        lnrow = cpool.tile([1, 4 * D], F32)
        for i, dr in enumerate([d_ln1g, d_ln1b, d_ln2g, d_ln2b]):
            nc.sync.dma_start(lnrow[0:1, i * D:(i + 1) * D], dr.ap()[:])
        epsc = cpool.tile([128, 1], F32)
        nc.vector.memset(epsc[:], EPS)
        lnbc = cpool.tile([128, 4, D], F32)
        for i in range(4):
            nc.gpsimd.partition_broadcast(lnbc[:, i, :], lnrow[0:1, i * D:(i + 1) * D])

        bpool = tc.alloc_tile_pool(name="bp", bufs=1)
        attnT = bpool.tile([128, DC, QTOK], F32R)     # attn^T
        srcq = bpool.tile([128, NTT, D], F32)
        perpool = tc.alloc_tile_pool(name="per", bufs=1)
        qT = perpool.tile([128, DC, QTOK], F32R)      # Q^T * scale (+bias)
        kT = perpool.tile([128, DC, T], F32R)         # K^T
        vN = perpool.tile([128, KB, D], F32R)         # V natural [t,d]
        nc.sync.dma_start(srcq[:], d_srcq.ap().rearrange("(tt p) d -> p tt d", p=128))

        # ================= Phase A: projections =================
        with tc.tile_pool(name="aw", bufs=1) as awp, \
             tc.tile_pool(name="pa1", bufs=3, space="PSUM") as pa1, \
             tc.tile_pool(name="pa2", bufs=2, space="PSUM") as pa2:
            srcT = awp.tile([128, DC, T], F32R)
            nc.sync.dma_start(srcT[:], d_srcT.ap().rearrange("(c p) t -> p c t", p=128))
            wq = awp.tile([128, DC, D], F32R)
            nc.sync.dma_start(wq[:], d_wqT.ap().rearrange("(c p) d -> p c d", p=128))
            wk = awp.tile([128, DC, D], F32R)
            nc.sync.dma_start(wk[:], d_wkT.ap().rearrange("(c p) d -> p c d", p=128))
            wv = awp.tile([128, DC, D], F32R)
            nc.sync.dma_start(wv[:], d_wvT.ap().rearrange("(c p) d -> p c d", p=128))

            # Q^T [D, 256] (pre-scaled by 1/sqrt(hd) on host, incl bias)
            for m in range(DC):
                ps = pa1.tile([128, 512], F32, tag="prj")
                for c in range(DC):
                    nc.tensor.matmul(ps[:, 0:QTOK], wq[:, c, m * 128:(m + 1) * 128],
                                     srcT[:, c, 0:QTOK], start=(c == 0), stop=(c == DC - 1))
                nc.scalar.activation(qT[:, m, :], ps[:, 0:QTOK], AF.Identity,
                                     bias=bqc[:, m:m + 1])
            # K^T [D, 1024]
            for m in range(DC):
                for kc in range(2):
                    ps = pa1.tile([128, 512], F32, tag="prj")
                    for c in range(DC):
                        nc.tensor.matmul(ps[:], wk[:, c, m * 128:(m + 1) * 128],
                                         srcT[:, c, kc * 512:(kc + 1) * 512],
                                         start=(c == 0), stop=(c == DC - 1))
                    nc.scalar.activation(kT[:, m, kc * 512:(kc + 1) * 512], ps[:],
                                         AF.Identity, bias=bkc[:, m:m + 1])
            # V natural [T, D]
            for t8 in range(KB):
                ps = pa2.tile([128, D], F32, tag="vprj")
                for (cs, ce) in chunks:
                    for c in range(DC):
                        nc.tensor.matmul(ps[:, cs:ce], srcT[:, c, t8 * 128:(t8 + 1) * 128],
                                         wv[:, c, cs:ce], start=(c == 0), stop=False)
                    nc.tensor.matmul(ps[:, cs:ce], ones1[0:1, :], bvr[0:1, cs:ce],
                                     start=False, stop=True)
                nc.scalar.copy(vN[:, t8, :], ps[:])

        # ================= Phase B: attention heads =================
        spool = tc.alloc_tile_pool(name="sp", bufs=2)
        with tc.tile_pool(name="pb1", bufs=2, space="PSUM") as pb1, \
             tc.tile_pool(name="pb2", bufs=1, space="PSUM") as pb2, \
             tc.tile_pool(name="st", bufs=2) as stp:
            for h in range(H):
                th, off = (64 * h) // 128, (64 * h) % 128
                sums = stp.tile([128, 2], F32, tag="sums")
                sqs = stp.tile([128, 2], F32, tag="sqs")
                mean = stp.tile([128, 2], F32, tag="mean")
                tm1 = stp.tile([128, 2], F32, tag="tm1")
                sig = stp.tile([128, 2], F32, tag="sig")
                sexp = stp.tile([128, 2], F32, tag="sexp")
                bexp = stp.tile([128, 2], F32, tag="bexp")
                den = stp.tile([128, 2], F32, tag="den")
                rden = stp.tile([128, 2], F32, tag="rden")
                S_list = []
                for qt in range(NTT):
                    S = pb1.tile([128, T], F32, tag="S")
                    S_list.append(S)
                    for kc in range(2):
                        nc.tensor.matmul(
                            S[:, kc * 512:(kc + 1) * 512],
                            qT[off:off + 64, th, qt * 128:(qt + 1) * 128],
                            kT[off:off + 64, th, kc * 512:(kc + 1) * 512],
                            start=True, stop=True)
                    nc.vector.reduce_sum(sums[:, qt:qt + 1], S[:], axis=AX.X)
                    sq = spool.tile([128, T], F32, tag="sqscr")
                    nc.scalar.activation(sq[:], S[:], AF.Square,
                                         accum_out=sqs[:, qt:qt + 1])
                # z-score params: s = gamma/(sqrt(var)+eps), b = -mean*s
                nc.vector.tensor_scalar_mul(mean[:], sums[:], 1.0 / T)
                nc.vector.tensor_tensor(tm1[:], sums[:], mean[:], op=ALU.mult)
                nc.vector.tensor_sub(tm1[:], sqs[:], tm1[:])
                nc.scalar.activation(sig[:], tm1[:], AF.Sqrt, scale=1.0 / (T - 1))
                nc.vector.tensor_scalar_add(sig[:], sig[:], EPS)
                nc.vector.reciprocal(sexp[:], sig[:])
                nc.vector.tensor_scalar_mul(sexp[:], sexp[:], float(gamma))
                nc.vector.scalar_tensor_tensor(bexp[:], mean[:], -1.0, sexp[:],
                                               op0=ALU.mult, op1=ALU.mult)
                PT = pb2.tile([128, KB, 128], F32, tag="PT")
                PTsb = spool.tile([128, NTT, KB, 128], F32R, tag="PTsb")
                for qt in range(NTT):
                    P = spool.tile([128, T], F32, tag="P")
                    nc.scalar.activation(P[:], S_list[qt][:], AF.Exp,
                                         bias=bexp[:, qt:qt + 1],
                                         scale=sexp[:, qt:qt + 1],
                                         accum_out=den[:, qt:qt + 1])
                    nc.vector.reciprocal(rden[:, qt:qt + 1], den[:, qt:qt + 1])
                    P2 = spool.tile([128, T], F32, tag="P2")
                    nc.scalar.mul(P2[:], P[:], rden[:, qt:qt + 1])
                    for kb in range(KB):
                        nc.tensor.transpose(PT[:, kb, :],
                                            P2[:, kb * 128:(kb + 1) * 128], ident[:])
                    nc.vector.tensor_copy(PTsb[:, qt], PT[:])
                aps = pb1.tile([64, QTOK], F32, tag="attnT")
                for kb in range(KB):
                    nc.tensor.matmul(aps[:], vN[:, kb, h * 64:(h + 1) * 64],
                                     PTsb[:, :, kb, :], start=(kb == 0),
                                     stop=(kb == KB - 1))
                nc.scalar.copy(attnT[off:off + 64, th, :], aps[:])
        spool.release()
        perpool.release()

        # ============ Phase C: Wo + LN1 + x^T + gate ============
        wpool = tc.alloc_tile_pool(name="cw", bufs=1)
        with tc.tile_pool(name="st2", bufs=2) as stp2:
          with tc.tile_pool(name="pc1", bufs=2, space="PSUM") as pc1:
              wo = wpool.tile([128, DC, D], F32R)
              nc.sync.dma_start(wo[:], d_woT.ap().rearrange("(c p) d -> p c d", p=128))
              wg = wpool.tile([128, DC, E], F32)
              nc.sync.dma_start(wg[:], d_wgT.ap().rearrange("(c p) e -> p c e", p=128))
              b2r = wpool.tile([1, E, D], F32R)
              nc.sync.dma_start(b2r[:], d_b2r.ap()[:])
              x_sb = wpool.tile([128, NTT, D], F32)       # post-LN1
              xT = wpool.tile([128, DC, NTT, 128], F32R)  # x^T
              xT32 = wpool.tile([128, DC, NTT, 128], F32)  # fp32 copy for gate
              comb = wpool.tile([128, NTT, E], F32)       # top-2 combine weights
              ffs = wpool.tile([128, NTT, D], F32)
              out_sb = wpool.tile([128, NTT, D], F32)

              def layer_norm(dst_ap, pre_ap, gb_idx):
                  s1 = stp2.tile([128, 1], F32, tag="s1")
                  q1 = stp2.tile([128, 1], F32, tag="q1")
                  mn = stp2.tile([128, 1], F32, tag="mn")
                  vv = stp2.tile([128, 1], F32, tag="vv")
                  rs = stp2.tile([128, 1], F32, tag="rs")
                  bb = stp2.tile([128, 1], F32, tag="bb")
                  xn = stp2.tile([128, D], F32, tag="xn")
                  sq2 = stp2.tile([128, D], F32, tag="sq2")
                  nc.vector.reduce_sum(s1[:], pre_ap, axis=AX.X)
                  nc.scalar.activation(sq2[:], pre_ap, AF.Square, accum_out=q1[:])
                  nc.vector.tensor_scalar_mul(mn[:], s1[:], 1.0 / D)
                  nc.vector.tensor_tensor(vv[:], mn[:], mn[:], op=ALU.mult)
                  nc.vector.scalar_tensor_tensor(vv[:], q1[:], 1.0 / D, vv[:],
                                                 op0=ALU.mult, op1=ALU.subtract)
                  sr = stp2.tile([128, 1], F32, tag="sr")
                  nc.scalar.activation(sr[:], vv[:], AF.Sqrt, bias=epsc[:])
                  nc.vector.reciprocal(rs[:], sr[:])
                  nc.vector.scalar_tensor_tensor(bb[:], mn[:], -1.0, rs[:],
                                                 op0=ALU.mult, op1=ALU.mult)
                  nc.scalar.activation(xn[:], pre_ap, AF.Identity,
                                       bias=bb[:], scale=rs[:])
                  nc.vector.tensor_tensor(xn[:], xn[:], lnbc[:, 2 * gb_idx, :], op=ALU.mult)
                  nc.vector.tensor_tensor(dst_ap, xn[:], lnbc[:, 2 * gb_idx + 1, :],
                                          op=ALU.add)

              for tt in range(NTT):
                  ps = pc1.tile([128, D], F32, tag="wo")
                  for (cs, ce) in chunks:
                      for c in range(DC):
                          nc.tensor.matmul(ps[:, cs:ce],
                                           attnT[:, c, tt * 128:(tt + 1) * 128],
                                           wo[:, c, cs:ce], start=(c == 0), stop=False)
                      nc.tensor.matmul(ps[:, cs:ce], ones1[0:1, :], bor[0:1, cs:ce],
                                       start=False, stop=True)
                  pre = stp2.tile([128, D], F32, tag="pre")
                  nc.vector.tensor_tensor(pre[:], ps[:], srcq[:, tt, :], op=ALU.add)
                  layer_norm(x_sb[:, tt, :], pre[:], 0)

              # x^T
              for c in range(DC):
                  xtp = pc1.tile([128, NTT, 128], F32, tag="xtp")
                  for tt in range(NTT):
                      nc.tensor.transpose(xtp[:, tt, :],
                                          x_sb[:, tt, c * 128:(c + 1) * 128], ident[:])
                  nc.vector.tensor_copy(xT[:, c], xtp[:])
                  nc.scalar.copy(xT32[:, c], xtp[:])

              # gate + top-2 combine
              for tt in range(NTT):
                  gp = pc1.tile([128, E], F32, tag="gate")
                  for c in range(DC):
                      nc.tensor.matmul(gp[:], xT32[:, c, tt, :], wg[:, c, :],
                                       start=(c == 0), stop=False)
                  nc.tensor.matmul(gp[:], ones1[0:1, :], bgr[0:1, :],
                                   start=False, stop=True)
                  mx = stp2.tile([128, 1], F32, tag="mx")
                  se = stp2.tile([128, 1], F32, tag="se")
                  eg = stp2.tile([128, E], F32, tag="eg")
                  pr = stp2.tile([128, E], F32, tag="pr")
                  m2 = stp2.tile([128, 1], F32, tag="m2")
                  kp = stp2.tile([128, E], F32, tag="kp")
                  nc.vector.reduce_max(mx[:], gp[:], axis=AX.X)
                  nc.vector.tensor_scalar_mul(mx[:], mx[:], -1.0)
                  nc.scalar.activation(eg[:], gp[:], AF.Exp, bias=mx[:], accum_out=se[:])
                  nc.vector.reciprocal(se[:], se[:])
                  nc.vector.tensor_scalar_mul(pr[:], eg[:], se[:])
                  nc.vector.reduce_max(mx[:], pr[:], axis=AX.X)
                  nc.vector.tensor_scalar(kp[:], pr[:], mx[:], None, op0=ALU.is_ge)
                  nc.vector.scalar_tensor_tensor(eg[:], kp[:], -1e9, pr[:],
                                                 op0=ALU.mult, op1=ALU.add)
                  nc.vector.reduce_max(m2[:], eg[:], axis=AX.X)
                  nc.vector.tensor_scalar(kp[:], pr[:], m2[:], None, op0=ALU.is_ge)
                  nc.vector.tensor_tensor(comb[:, tt, :], pr[:], kp[:], op=ALU.mult)

          # ============ Phase D: MoE experts ============
          with tc.tile_pool(name="mw", bufs=3) as mwp, \
               tc.tile_pool(name="ffp", bufs=1) as ffp, \
               tc.tile_pool(name="pd1", bufs=3, space="PSUM") as pd1, \
               tc.tile_pool(name="pd2", bufs=1, space="PSUM") as pd2:
              ff_t = [ffp.tile([128, NTT, D], F32, tag=f"ff{e}", name=f"ff{e}") for e in range(E)]
              for e in range(E):
                  yps = [pd2.tile([128, D], F32, tag=f"y{tt}", name=f"y{tt}") for tt in range(NTT)]
                  for ft in range(FT):
                      w1t = mwp.tile([128, DC, 128], F32R, tag="w1t")
                      nc.sync.dma_start(
                          w1t[:],
                          d_w1.ap()[e, :, ft * 128:(ft + 1) * 128]
                          .rearrange("(c p) f -> p c f", p=128))
                      w2t = mwp.tile([128, D], F32R, tag="w2t")
                      nc.sync.dma_start(w2t[:], d_w2.ap()[e, ft * 128:(ft + 1) * 128, :])
                      hp = pd1.tile([128, QTOK], F32, tag="hps")
                      for c in range(DC):
                          nc.tensor.matmul(hp[:], w1t[:, c, :], xT[:, c],
                                           start=(c == 0), stop=(c == DC - 1))
                      hsb = mwp.tile([128, QTOK], F32R, tag="hsb")
                      nc.scalar.activation(hsb[:], hp[:], AF.Relu,
                                           bias=b1c[:, e, ft:ft + 1])
                      for tt in range(NTT):
                          for (cs, ce) in chunks:
                              nc.tensor.matmul(yps[tt][:, cs:ce],
                                               hsb[:, tt * 128:(tt + 1) * 128],
                                               w2t[:, cs:ce],
                                               start=(ft == 0), stop=False)
                  for tt in range(NTT):
                      for (cs, ce) in chunks:
                          nc.tensor.matmul(yps[tt][:, cs:ce], ones1[0:1, :],
                                           b2r[0:1, e, cs:ce], start=False, stop=True)
                      nc.scalar.mul(ff_t[e][:, tt, :], yps[tt][:],
                                    comb[:, tt, e:e + 1])
              nc.vector.tensor_tensor(ff_t[0][:], ff_t[0][:], ff_t[1][:], op=ALU.add)
              nc.vector.tensor_tensor(ff_t[2][:], ff_t[2][:], ff_t[3][:], op=ALU.add)
              nc.vector.tensor_tensor(ffs[:], ff_t[0][:], ff_t[2][:], op=ALU.add)

              for tt in range(NTT):
                  pre2 = stp2.tile([128, D], F32, tag="pre")
                  nc.vector.tensor_tensor(pre2[:], x_sb[:, tt, :], ffs[:, tt, :],
                                          op=ALU.add)
                  layer_norm(out_sb[:, tt, :], pre2[:], 1)
              nc.sync.dma_start(
                  d_out.ap().rearrange("(tt p) d -> p tt d", p=128), out_sb[:])
        wpool.release()
        bpool.release()
        cpool.release()

    nc.compile()
    return nc


def _prep(inputs):
    f = lambda a: np.ascontiguousarray(np.asarray(a, dtype=np.float32))
    src = f(inputs["src"])
    scale = (D // H) ** -0.5
    common = {
        "wqT": f(inputs["Wq"]).T.copy() * scale,
        "wkT": f(inputs["Wk"]).T.copy(),
        "wvT": f(inputs["Wv"]).T.copy(),
        "woT": f(inputs["Wo"]).T.copy(),
        "bqc": (f(inputs["bq"]) * scale).reshape(DC, 128).T.copy(),
        "bkc": f(inputs["bk"]).reshape(DC, 128).T.copy(),
        "bvr": f(inputs["bv"]).reshape(1, D),
        "bor": f(inputs["bo"]).reshape(1, D),
        "ln1g": f(inputs["ln1_g"]).reshape(1, D),
        "ln1b": f(inputs["ln1_b"]).reshape(1, D),
        "ln2g": f(inputs["ln2_g"]).reshape(1, D),
        "ln2b": f(inputs["ln2_b"]).reshape(1, D),
        "wgT": f(inputs["Wg"]).T.copy(),
        "bgr": f(inputs["bg"]).reshape(1, E),
        "w1": f(inputs["W1"]),
        "b1c": np.ascontiguousarray(
            f(inputs["b1"]).reshape(E, FT, 128).transpose(0, 2, 1)),
        "w2": f(inputs["W2"]),
        "b2r": f(inputs["b2"]).reshape(1, E, D),
        "ident": np.eye(128, dtype=np.float32),
        "ones_r": np.ones((1, 128), dtype=np.float32),
    }
    in_maps = []
    for c in range(NCORES):
        b, qq = c // 4, c % 4
        m = dict(common)
        # rotate key/value token axis so this core's quarter sits at cols 0:256
        m["srcT"] = np.ascontiguousarray(np.roll(src[b].T, -qq * QTOK, axis=1))
        m["srcq"] = np.ascontiguousarray(src[b, qq * QTOK:(qq + 1) * QTOK])
        in_maps.append(m)
    return in_maps


def kernel(**inputs):
    global LAST_RESULT
    gamma = float(np.asarray(inputs["gamma"]))
    key = round(gamma, 9)
    if key not in _cache:
        _cache[key] = _build(gamma)
    nc = _cache[key]
    in_maps = _prep(inputs)
    trace = bool(os.environ.get("KERNEL_TRACE"))
    try:
        res = run_bass_kernel_spmd(nc, in_maps, list(range(NCORES)), trace=trace)
    except ModuleNotFoundError:
        res = run_bass_kernel_spmd(nc, in_maps, list(range(NCORES)), trace=False)
    LAST_RESULT = res
    out = np.empty((B, T, D), dtype=np.float32)
    for c in range(NCORES):
        b, qq = c // 4, c % 4
        out[b, qq * QTOK:(qq + 1) * QTOK] = res.results[c]["out"]
    return out



# revision 27
# speedup vs baseline: 1.1399x; 1.1399x over previous
"""Trainium2 Bass kernel for CustomTransformerEncoderMoELayer (moe_routing).

Sharding: 8 cores = 2 batches x 4 query-quarters. Each core:
  - projects K^T, V for its full batch (replicated within batch group),
  - computes attention rows for its 256 query tokens (z-score + softmax),
  - residual + LN1, then dense 4-expert MoE (top-2 combine weights) on its
    256 tokens, residual + LN2.
No cross-core communication; host only shards inputs / concatenates outputs.
"""
import os
import numpy as np
import ml_dtypes

import concourse.bacc as bacc
import concourse.mybir as mybir
import concourse.tile as tile
from concourse.bass_utils import run_bass_kernel_spmd

F32 = mybir.dt.float32
F32R = mybir.dt.float32r
BF16 = mybir.dt.bfloat16
AF = mybir.ActivationFunctionType
ALU = mybir.AluOpType
AX = mybir.AxisListType

B, T, D, FFD, E, H = 2, 1024, 768, 3072, 4, 12
HD = D // H          # 64
QTOK = 256           # query tokens per core
NCORES = 8
DC = D // 128        # 6 chunks of contraction dim
FT = FFD // 128      # 24 FF tiles
NTT = QTOK // 128    # 2 token tiles
KB = T // 128        # 8 key blocks
EPS = 1e-5

_cache = {}
LAST_RESULT = None


def _build(gamma: float):
    nc = bacc.Bacc("TRN2", target_bir_lowering=False, debug=False,
                   num_devices=NCORES)

    # ---- DRAM I/O ----
    d_srcT = nc.dram_tensor("srcT", [D, T], F32R, kind="ExternalInput")
    d_srcq = nc.dram_tensor("srcq", [QTOK, D], F32, kind="ExternalInput")
    d_wqT = nc.dram_tensor("wqT", [D, D], F32R, kind="ExternalInput")
    d_wkT = nc.dram_tensor("wkT", [D, D], F32R, kind="ExternalInput")
    d_wvT = nc.dram_tensor("wvT", [D, D], F32R, kind="ExternalInput")
    d_woT = nc.dram_tensor("woT", [D, D], F32R, kind="ExternalInput")
    d_bqc = nc.dram_tensor("bqc", [128, DC], F32, kind="ExternalInput")
    d_bkc = nc.dram_tensor("bkc", [128, DC], F32, kind="ExternalInput")
    d_bvr = nc.dram_tensor("bvr", [1, D], F32R, kind="ExternalInput")
    d_bor = nc.dram_tensor("bor", [1, D], F32R, kind="ExternalInput")
    d_ln1g = nc.dram_tensor("ln1g", [1, D], F32, kind="ExternalInput")
    d_ln1b = nc.dram_tensor("ln1b", [1, D], F32, kind="ExternalInput")
    d_ln2g = nc.dram_tensor("ln2g", [1, D], F32, kind="ExternalInput")
    d_ln2b = nc.dram_tensor("ln2b", [1, D], F32, kind="ExternalInput")
    d_wgT = nc.dram_tensor("wgT", [D, E], F32, kind="ExternalInput")
    d_bgr = nc.dram_tensor("bgr", [1, E], F32R, kind="ExternalInput")
    d_w1 = nc.dram_tensor("w1", [E, D, FFD], BF16, kind="ExternalInput")
    d_b1c = nc.dram_tensor("b1c", [E, 128, FT], F32, kind="ExternalInput")
    d_w2 = nc.dram_tensor("w2", [E, FFD, D], BF16, kind="ExternalInput")
    d_b2r = nc.dram_tensor("b2r", [1, E, D], F32R, kind="ExternalInput")
    d_ident = nc.dram_tensor("ident", [128, 128], F32, kind="ExternalInput")
    d_ones = nc.dram_tensor("ones_r", [1, 128], F32R, kind="ExternalInput")
    d_out = nc.dram_tensor("out", [QTOK, D], F32, kind="ExternalOutput")

    chunks = [(0, 512), (512, 768)]  # free-dim chunks of D for matmul N<=512

    with tile.TileContext(nc) as tc:
        # ----- persistent pools -----
        cpool = tc.alloc_tile_pool(name="const", bufs=1)
        ident = cpool.tile([128, 128], F32)
        nc.sync.dma_start(ident[:], d_ident.ap()[:])
        ones1 = cpool.tile([1, 128], F32R)
        nc.sync.dma_start(ones1[:], d_ones.ap()[:])
        bqc = cpool.tile([128, DC], F32)
        nc.sync.dma_start(bqc[:], d_bqc.ap()[:])
        bkc = cpool.tile([128, DC], F32)
        nc.sync.dma_start(bkc[:], d_bkc.ap()[:])
        bvr = cpool.tile([1, D], F32R)
        nc.sync.dma_start(bvr[:], d_bvr.ap()[:])
        bor = cpool.tile([1, D], F32R)
        nc.sync.dma_start(bor[:], d_bor.ap()[:])
        bgr = cpool.tile([1, E], F32R)
        nc.sync.dma_start(bgr[:], d_bgr.ap()[:])
        b1c = cpool.tile([128, E, FT], F32)
        nc.sync.dma_start(b1c[:], d_b1c.ap().rearrange("e p c -> p e c"))
        lnrow = cpool.tile([1, 4 * D], F32)
        for i, dr in enumerate([d_ln1g, d_ln1b, d_ln2g, d_ln2b]):
            nc.sync.dma_start(lnrow[0:1, i * D:(i + 1) * D], dr.ap()[:])
        epsc = cpool.tile([128, 1], F32)
        nc.vector.memset(epsc[:], EPS)
        lnbc = cpool.tile([128, 4, D], F32)
        for i in range(4):
            nc.gpsimd.partition_broadcast(lnbc[:, i, :], lnrow[0:1, i * D:(i + 1) * D])

        bpool = tc.alloc_tile_pool(name="bp", bufs=1)
        attnT = bpool.tile([128, DC, QTOK], F32R)     # attn^T
        srcq = bpool.tile([128, NTT, D], F32)
        perpool = tc.alloc_tile_pool(name="per", bufs=1)
        qT = perpool.tile([128, DC, QTOK], F32R)      # Q^T * scale (+bias)
        kT = perpool.tile([128, DC, T], F32R)         # K^T
        vN = perpool.tile([128, KB, D], F32R)         # V natural [t,d]
        nc.sync.dma_start(srcq[:], d_srcq.ap().rearrange("(tt p) d -> p tt d", p=128))

        # ================= Phase A: projections =================
        with tc.tile_pool(name="aw", bufs=1) as awp, \
             tc.tile_pool(name="pa1", bufs=3, space="PSUM") as pa1, \
             tc.tile_pool(name="pa2", bufs=2, space="PSUM") as pa2:
            srcT = awp.tile([128, DC, T], F32R)
            nc.sync.dma_start(srcT[:], d_srcT.ap().rearrange("(c p) t -> p c t", p=128))
            wq = awp.tile([128, DC, D], F32R)
            nc.sync.dma_start(wq[:], d_wqT.ap().rearrange("(c p) d -> p c d", p=128))
            wk = awp.tile([128, DC, D], F32R)
            nc.sync.dma_start(wk[:], d_wkT.ap().rearrange("(c p) d -> p c d", p=128))
            wv = awp.tile([128, DC, D], F32R)
            nc.sync.dma_start(wv[:], d_wvT.ap().rearrange("(c p) d -> p c d", p=128))

            # Q^T [D, 256] (pre-scaled by 1/sqrt(hd) on host, incl bias)
            for m in range(DC):
                ps = pa1.tile([128, 512], F32, tag="prj")
                for c in range(DC):
                    nc.tensor.matmul(ps[:, 0:QTOK], wq[:, c, m * 128:(m + 1) * 128],
                                     srcT[:, c, 0:QTOK], start=(c == 0), stop=(c == DC - 1))
                nc.scalar.activation(qT[:, m, :], ps[:, 0:QTOK], AF.Identity,
                                     bias=bqc[:, m:m + 1])
            # K^T [D, 1024]
            for m in range(DC):
                for kc in range(2):
                    ps = pa1.tile([128, 512], F32, tag="prj")
                    for c in range(DC):
                        nc.tensor.matmul(ps[:], wk[:, c, m * 128:(m + 1) * 128],
                                         srcT[:, c, kc * 512:(kc + 1) * 512],
                                         start=(c == 0), stop=(c == DC - 1))
                    nc.scalar.activation(kT[:, m, kc * 512:(kc + 1) * 512], ps[:],
                                         AF.Identity, bias=bkc[:, m:m + 1])
            # V natural [T, D]
            for t8 in range(KB):
                ps = pa2.tile([128, D], F32, tag="vprj")
                for (cs, ce) in chunks:
                    for c in range(DC):
                        nc.tensor.matmul(ps[:, cs:ce], srcT[:, c, t8 * 128:(t8 + 1) * 128],
                                         wv[:, c, cs:ce], start=(c == 0), stop=False)
                    nc.tensor.matmul(ps[:, cs:ce], ones1[0:1, :], bvr[0:1, cs:ce],
                                     start=False, stop=True)
                nc.scalar.copy(vN[:, t8, :], ps[:])

        # ================= Phase B: attention heads =================
        spool = tc.alloc_tile_pool(name="sp", bufs=2)
        with tc.tile_pool(name="pb1", bufs=2, space="PSUM") as pb1, \
             tc.tile_pool(name="pb2", bufs=1, space="PSUM") as pb2, \
             tc.tile_pool(name="st", bufs=2) as stp:
            for h in range(H):
                th, off = (64 * h) // 128, (64 * h) % 128
                sums = stp.tile([128, 2], F32, tag="sums")
                sqs = stp.tile([128, 2], F32, tag="sqs")
                mean = stp.tile([128, 2], F32, tag="mean")
                tm1 = stp.tile([128, 2], F32, tag="tm1")
                sig = stp.tile([128, 2], F32, tag="sig")
                sexp = stp.tile([128, 2], F32, tag="sexp")
                bexp = stp.tile([128, 2], F32, tag="bexp")
                den = stp.tile([128, 2], F32, tag="den")
                rden = stp.tile([128, 2], F32, tag="rden")
                S_list = []
                for qt in range(NTT):
                    S = pb1.tile([128, T], F32, tag="S")
                    S_list.append(S)
                    for kc in range(2):
                        nc.tensor.matmul(
                            S[:, kc * 512:(kc + 1) * 512],
                            qT[off:off + 64, th, qt * 128:(qt + 1) * 128],
                            kT[off:off + 64, th, kc * 512:(kc + 1) * 512],
                            start=True, stop=True)
                    nc.vector.reduce_sum(sums[:, qt:qt + 1], S[:], axis=AX.X)
                    sq = spool.tile([128, T], F32, tag="sqscr")
                    nc.scalar.activation(sq[:], S[:], AF.Square,
                                         accum_out=sqs[:, qt:qt + 1])
                # z-score params: s = gamma/(sqrt(var)+eps), b = -mean*s
                nc.vector.tensor_scalar_mul(mean[:], sums[:], 1.0 / T)
                nc.vector.tensor_tensor(tm1[:], sums[:], mean[:], op=ALU.mult)
                nc.vector.tensor_sub(tm1[:], sqs[:], tm1[:])
                nc.scalar.activation(sig[:], tm1[:], AF.Sqrt, scale=1.0 / (T - 1))
                nc.vector.tensor_scalar_add(sig[:], sig[:], EPS)
                nc.vector.reciprocal(sexp[:], sig[:])
                nc.vector.tensor_scalar_mul(sexp[:], sexp[:], float(gamma))
                nc.vector.scalar_tensor_tensor(bexp[:], mean[:], -1.0, sexp[:],
                                               op0=ALU.mult, op1=ALU.mult)
                PT = pb2.tile([128, KB, 128], F32, tag="PT")
                PTsb = spool.tile([128, NTT, KB, 128], F32R, tag="PTsb")
                for qt in range(NTT):
                    P = spool.tile([128, T], F32, tag="P")
                    nc.scalar.activation(P[:], S_list[qt][:], AF.Exp,
                                         bias=bexp[:, qt:qt + 1],
                                         scale=sexp[:, qt:qt + 1],
                                         accum_out=den[:, qt:qt + 1])
                    nc.vector.reciprocal(rden[:, qt:qt + 1], den[:, qt:qt + 1])
                    P2 = spool.tile([128, T], F32, tag="P2")
                    nc.scalar.mul(P2[:], P[:], rden[:, qt:qt + 1])
                    for kb in range(KB):
                        nc.tensor.transpose(PT[:, kb, :],
                                            P2[:, kb * 128:(kb + 1) * 128], ident[:])
                    nc.vector.tensor_copy(PTsb[:, qt], PT[:])
                aps = pb1.tile([64, QTOK], F32, tag="attnT")
                for kb in range(KB):
                    nc.tensor.matmul(aps[:], vN[:, kb, h * 64:(h + 1) * 64],
                                     PTsb[:, :, kb, :], start=(kb == 0),
                                     stop=(kb == KB - 1))
                nc.scalar.copy(attnT[off:off + 64, th, :], aps[:])
        spool.release()
        perpool.release()

        # ============ Phase C: Wo + LN1 + x^T + gate ============
        wpool = tc.alloc_tile_pool(name="cw", bufs=1)
        with tc.tile_pool(name="st2", bufs=2) as stp2:
          with tc.tile_pool(name="pc1", bufs=2, space="PSUM") as pc1:
              wo = wpool.tile([128, DC, D], F32R)
              nc.sync.dma_start(wo[:], d_woT.ap().rearrange("(c p) d -> p c d", p=128))
              wg = wpool.tile([128, DC, E], F32)
              nc.sync.dma_start(wg[:], d_wgT.ap().rearrange("(c p) e -> p c e", p=128))
              b2r = wpool.tile([1, E, D], F32R)
              nc.sync.dma_start(b2r[:], d_b2r.ap()[:])
              x_sb = wpool.tile([128, NTT, D], F32)       # post-LN1
              xT = wpool.tile([128, DC, NTT, 128], BF16)  # x^T
              xT32 = wpool.tile([128, DC, NTT, 128], F32)  # fp32 copy for gate
              comb = wpool.tile([128, NTT, E], F32)       # top-2 combine weights
              ffs = wpool.tile([128, NTT, D], F32)
              out_sb = wpool.tile([128, NTT, D], F32)

              def layer_norm(dst_ap, pre_ap, gb_idx):
                  s1 = stp2.tile([128, 1], F32, tag="s1")
                  q1 = stp2.tile([128, 1], F32, tag="q1")
                  mn = stp2.tile([128, 1], F32, tag="mn")
                  vv = stp2.tile([128, 1], F32, tag="vv")
                  rs = stp2.tile([128, 1], F32, tag="rs")
                  bb = stp2.tile([128, 1], F32, tag="bb")
                  xn = stp2.tile([128, D], F32, tag="xn")
                  sq2 = stp2.tile([128, D], F32, tag="sq2")
                  nc.vector.reduce_sum(s1[:], pre_ap, axis=AX.X)
                  nc.scalar.activation(sq2[:], pre_ap, AF.Square, accum_out=q1[:])
                  nc.vector.tensor_scalar_mul(mn[:], s1[:], 1.0 / D)
                  nc.vector.tensor_tensor(vv[:], mn[:], mn[:], op=ALU.mult)
                  nc.vector.scalar_tensor_tensor(vv[:], q1[:], 1.0 / D, vv[:],
                                                 op0=ALU.mult, op1=ALU.subtract)
                  sr = stp2.tile([128, 1], F32, tag="sr")
                  nc.scalar.activation(sr[:], vv[:], AF.Sqrt, bias=epsc[:])
                  nc.vector.reciprocal(rs[:], sr[:])
                  nc.vector.scalar_tensor_tensor(bb[:], mn[:], -1.0, rs[:],
                                                 op0=ALU.mult, op1=ALU.mult)
                  nc.scalar.activation(xn[:], pre_ap, AF.Identity,
                                       bias=bb[:], scale=rs[:])
                  nc.vector.tensor_tensor(xn[:], xn[:], lnbc[:, 2 * gb_idx, :], op=ALU.mult)
                  nc.vector.tensor_tensor(dst_ap, xn[:], lnbc[:, 2 * gb_idx + 1, :],
                                          op=ALU.add)

              for tt in range(NTT):
                  ps = pc1.tile([128, D], F32, tag="wo")
                  for (cs, ce) in chunks:
                      for c in range(DC):
                          nc.tensor.matmul(ps[:, cs:ce],
                                           attnT[:, c, tt * 128:(tt + 1) * 128],
                                           wo[:, c, cs:ce], start=(c == 0), stop=False)
                      nc.tensor.matmul(ps[:, cs:ce], ones1[0:1, :], bor[0:1, cs:ce],
                                       start=False, stop=True)
                  pre = stp2.tile([128, D], F32, tag="pre")
                  nc.vector.tensor_tensor(pre[:], ps[:], srcq[:, tt, :], op=ALU.add)
                  layer_norm(x_sb[:, tt, :], pre[:], 0)

              # x^T
              for c in range(DC):
                  xtp = pc1.tile([128, NTT, 128], F32, tag="xtp")
                  for tt in range(NTT):
                      nc.tensor.transpose(xtp[:, tt, :],
                                          x_sb[:, tt, c * 128:(c + 1) * 128], ident[:])
                  nc.vector.tensor_copy(xT[:, c], xtp[:])
                  nc.scalar.copy(xT32[:, c], xtp[:])

              # gate + top-2 combine
              for tt in range(NTT):
                  gp = pc1.tile([128, E], F32, tag="gate")
                  for c in range(DC):
                      nc.tensor.matmul(gp[:], xT32[:, c, tt, :], wg[:, c, :],
                                       start=(c == 0), stop=False)
                  nc.tensor.matmul(gp[:], ones1[0:1, :], bgr[0:1, :],
                                   start=False, stop=True)
                  mx = stp2.tile([128, 1], F32, tag="mx")
                  se = stp2.tile([128, 1], F32, tag="se")
                  eg = stp2.tile([128, E], F32, tag="eg")
                  pr = stp2.tile([128, E], F32, tag="pr")
                  m2 = stp2.tile([128, 1], F32, tag="m2")
                  kp = stp2.tile([128, E], F32, tag="kp")
                  nc.vector.reduce_max(mx[:], gp[:], axis=AX.X)
                  nc.vector.tensor_scalar_mul(mx[:], mx[:], -1.0)
                  nc.scalar.activation(eg[:], gp[:], AF.Exp, bias=mx[:], accum_out=se[:])
                  nc.vector.reciprocal(se[:], se[:])
                  nc.vector.tensor_scalar_mul(pr[:], eg[:], se[:])
                  nc.vector.reduce_max(mx[:], pr[:], axis=AX.X)
                  nc.vector.tensor_scalar(kp[:], pr[:], mx[:], None, op0=ALU.is_ge)
                  nc.vector.scalar_tensor_tensor(eg[:], kp[:], -1e9, pr[:],
                                                 op0=ALU.mult, op1=ALU.add)
                  nc.vector.reduce_max(m2[:], eg[:], axis=AX.X)
                  nc.vector.tensor_scalar(kp[:], pr[:], m2[:], None, op0=ALU.is_ge)
                  nc.vector.tensor_tensor(comb[:, tt, :], pr[:], kp[:], op=ALU.mult)

          # ============ Phase D: MoE experts ============
          with tc.tile_pool(name="mw", bufs=8) as mwp, \
               tc.tile_pool(name="ffp", bufs=1) as ffp, \
               tc.tile_pool(name="pd1", bufs=3, space="PSUM") as pd1, \
               tc.tile_pool(name="pd2", bufs=1, space="PSUM") as pd2:
              ff_t = [ffp.tile([128, NTT, D], F32, tag=f"ff{e}", name=f"ff{e}") for e in range(E)]
              for e in range(E):
                  yps = [pd2.tile([128, D], F32, tag=f"y{tt}", name=f"y{tt}") for tt in range(NTT)]
                  for ft in range(FT):
                      w1t = mwp.tile([128, DC, 128], BF16, tag="w1t")
                      nc.sync.dma_start(
                          w1t[:],
                          d_w1.ap()[e, :, ft * 128:(ft + 1) * 128]
                          .rearrange("(c p) f -> p c f", p=128))
                      w2t = mwp.tile([128, D], BF16, tag="w2t")
                      nc.sync.dma_start(w2t[:], d_w2.ap()[e, ft * 128:(ft + 1) * 128, :])
                      hp = pd1.tile([128, QTOK], F32, tag="hps")
                      for c in range(DC):
                          nc.tensor.matmul(hp[:], w1t[:, c, :], xT[:, c],
                                           start=(c == 0), stop=(c == DC - 1))
                      hsb = mwp.tile([128, QTOK], BF16, tag="hsb")
                      nc.scalar.activation(hsb[:], hp[:], AF.Relu,
                                           bias=b1c[:, e, ft:ft + 1])
                      for tt in range(NTT):
                          for (cs, ce) in chunks:
                              nc.tensor.matmul(yps[tt][:, cs:ce],
                                               hsb[:, tt * 128:(tt + 1) * 128],
                                               w2t[:, cs:ce],
                                               start=(ft == 0), stop=False)
                  for tt in range(NTT):
                      for (cs, ce) in chunks:
                          nc.tensor.matmul(yps[tt][:, cs:ce], ones1[0:1, :],
                                           b2r[0:1, e, cs:ce], start=False, stop=True)
                      nc.scalar.mul(ff_t[e][:, tt, :], yps[tt][:],
                                    comb[:, tt, e:e + 1])
              nc.vector.tensor_tensor(ff_t[0][:], ff_t[0][:], ff_t[1][:], op=ALU.add)
              nc.vector.tensor_tensor(ff_t[2][:], ff_t[2][:], ff_t[3][:], op=ALU.add)
              nc.vector.tensor_tensor(ffs[:], ff_t[0][:], ff_t[2][:], op=ALU.add)

              for tt in range(NTT):
                  pre2 = stp2.tile([128, D], F32, tag="pre")
                  nc.vector.tensor_tensor(pre2[:], x_sb[:, tt, :], ffs[:, tt, :],
                                          op=ALU.add)
                  layer_norm(out_sb[:, tt, :], pre2[:], 1)
              nc.sync.dma_start(
                  d_out.ap().rearrange("(tt p) d -> p tt d", p=128), out_sb[:])
        wpool.release()
        bpool.release()
        cpool.release()

    nc.compile()
    return nc


def _prep(inputs):
    f = lambda a: np.ascontiguousarray(np.asarray(a, dtype=np.float32))
    src = f(inputs["src"])
    scale = (D // H) ** -0.5
    common = {
        "wqT": f(inputs["Wq"]).T.copy() * scale,
        "wkT": f(inputs["Wk"]).T.copy(),
        "wvT": f(inputs["Wv"]).T.copy(),
        "woT": f(inputs["Wo"]).T.copy(),
        "bqc": (f(inputs["bq"]) * scale).reshape(DC, 128).T.copy(),
        "bkc": f(inputs["bk"]).reshape(DC, 128).T.copy(),
        "bvr": f(inputs["bv"]).reshape(1, D),
        "bor": f(inputs["bo"]).reshape(1, D),
        "ln1g": f(inputs["ln1_g"]).reshape(1, D),
        "ln1b": f(inputs["ln1_b"]).reshape(1, D),
        "ln2g": f(inputs["ln2_g"]).reshape(1, D),
        "ln2b": f(inputs["ln2_b"]).reshape(1, D),
        "wgT": f(inputs["Wg"]).T.copy(),
        "bgr": f(inputs["bg"]).reshape(1, E),
        "w1": np.ascontiguousarray(np.asarray(inputs["W1"], dtype=ml_dtypes.bfloat16)),
        "b1c": np.ascontiguousarray(
            f(inputs["b1"]).reshape(E, FT, 128).transpose(0, 2, 1)),
        "w2": np.ascontiguousarray(np.asarray(inputs["W2"], dtype=ml_dtypes.bfloat16)),
        "b2r": f(inputs["b2"]).reshape(1, E, D),
        "ident": np.eye(128, dtype=np.float32),
        "ones_r": np.ones((1, 128), dtype=np.float32),
    }
    in_maps = []
    for c in range(NCORES):
        b, qq = c // 4, c % 4
        m = dict(common)
        # rotate key/value token axis so this core's quarter sits at cols 0:256
        m["srcT"] = np.ascontiguousarray(np.roll(src[b].T, -qq * QTOK, axis=1))
        m["srcq"] = np.ascontiguousarray(src[b, qq * QTOK:(qq + 1) * QTOK])
        in_maps.append(m)
    return in_maps


def kernel(**inputs):
    global LAST_RESULT
    gamma = float(np.asarray(inputs["gamma"]))
    key = round(gamma, 9)
    if key not in _cache:
        _cache[key] = _build(gamma)
    nc = _cache[key]
    in_maps = _prep(inputs)
    trace = bool(os.environ.get("KERNEL_TRACE"))
    try:
        res = run_bass_kernel_spmd(nc, in_maps, list(range(NCORES)), trace=trace)
    except ModuleNotFoundError:
        res = run_bass_kernel_spmd(nc, in_maps, list(range(NCORES)), trace=False)
    LAST_RESULT = res
    out = np.empty((B, T, D), dtype=np.float32)
    for c in range(NCORES):
        b, qq = c // 4, c % 4
        out[b, qq * QTOK:(qq + 1) * QTOK] = res.results[c]["out"]
    return out

